# revision 6
# baseline (speedup 1.0000x reference)
"""Merged dilated-group conv2d (4 groups, dil 1/6/12/18) for TRN2, 8 cores.

Sharding: data-parallel over batch (8 images -> 8 cores).

Per-core kernel:
  conv-as-matmul, contraction K = 48 = (3 kh-taps x 16 in-ch) on SBUF
  partitions; the 3 kw-taps are PSUM-accumulating matmuls reading the same
  SBUF row-window at shifted free-dim offsets. All 4 dilation groups run
  concurrently on disjoint PE subarray quadrants via tile_position
  (row-tiles {0,64} x col-tiles {0,32,64,96}); M=32 per group (16 real
  out-channels + 16 zero cols) so the 4 groups tile a PSUM bank across all
  128 partitions and each output row is evicted with a single
  128-partition op (bias fused), alternating ACT/DVE.

  Input rows stream through per-group SBUF rings of zero-padded row
  windows (Wp = W + 2d).  Two ring schemes per group:
    'rep'  : slot per output row holds the 3 kh rows (rows read from HBM
             3x, deep ring, fully decoupled pipelining)
    'mod3' : row r lives once in partition block floor(r/d) mod 3, slot
             r mod d; the kh->block rotation is absorbed into 3
             precomputed weight variants (rows read from HBM 1x)
"""

import os
import numpy as np

H = 320
W = 320
B = 8
C = 64
G = 4
ICG = 16  # in-channels per group
OCG = 16  # out-channels per group
DIL = (1, 6, 12, 18)
NCORES = 8

# group g -> (row-tile base RB, slot s within row tile, col-tile base)
GROUP_POS = {
    0: (0, 0, 0),
    1: (64, 0, 32),
    2: (0, 1, 64),
    3: (64, 1, 96),
}

# per-group input scheme: 'rep' (3x HBM reads) or 'mod3' (1x HBM reads)
SCHEMES = ("rep", "mod3", "mod3", "mod3")

S_REP = 16   # rep-ring slots (must be multiple of CH_REP)
CH_REP = 8   # rep-ring DMA chunk (output rows per in-DMA)
R_OUT = 16   # output staging rows per out-DMA block
NB = 8       # psum banks in rotation


def _prep_weights(weight: np.ndarray, schemes) -> np.ndarray:
    """-> [G, 48, 3(variant), 3(kw), 32] lhsT tiles.

    lhsT row q = 16*b + ic (b = partition block), col j = out-channel
    (j<16) or zero pad (j>=16).  Block b holds kh = b for 'rep'; for
    'mod3' output row rho with v = floor(rho/d) mod 3 reads kh =
    (b - v + 1) mod 3 from block b.
    """
    wt = np.zeros((G, 48, 6, 3, 32), np.float32)
    for g in range(G):
        for v in range(3):
            for b in range(3):
                kh = b if schemes[g] == "rep" else (b - v + 1) % 3
                for ic in range(ICG):
                    # weight[oc_global, ic, kh, kw] -> tile[16b+ic, v, kw, oc]
                    wt[g, 16 * b + ic, v, :, :OCG] = weight[
                        16 * g : 16 * g + OCG, ic, kh, :
                    ].T
            # bottom-edge variant v+3: kh=2 block zeroed (reads stale slots)
            bstar = 2 if schemes[g] == "rep" else (v + 1) % 3
            wt[g, :, v + 3] = wt[g, :, v]
            wt[g, 16 * bstar : 16 * bstar + ICG, v + 3] = 0.0
    return wt


def _prep_bias(bias: np.ndarray) -> np.ndarray:
    bp = np.zeros((128, 1), np.float32)
    for g in range(G):
        _, _, cb = GROUP_POS[g]
        bp[cb : cb + OCG, 0] = bias[16 * g : 16 * g + OCG]
    return bp


def build_module(h=H, schemes=SCHEMES, s_rep=S_REP, ch_rep=CH_REP, r_out=R_OUT):
    import concourse.bass as bass
    import concourse.tile as tile
    from concourse import bacc, mybir

    f32 = mybir.dt.float32
    Wp = [W + 2 * d for d in DIL]

    nc = bacc.Bacc("TRN2", target_bir_lowering=False, debug=False)
    x_d = nc.dram_tensor("x", [C, h, W], f32, kind="ExternalInput")
    wt_d = nc.dram_tensor("wt", [G, 48, 6, 3, 32], f32, kind="ExternalInput")
    bias_d = nc.dram_tensor("biasp", [128, 1], f32, kind="ExternalInput")
    out_d = nc.dram_tensor("out", [C, h, W], f32, kind="ExternalOutput")

    with tile.TileContext(nc) as tc:
        # ---- persistent SBUF/PSUM ----
        rings = []
        for g in range(G):
            nslot = s_rep if schemes[g] == "rep" else DIL[g]
            rings.append(
                nc.alloc_sbuf_tensor(f"ring{g}", [128, nslot, Wp[g]], f32)
            )
        wts_sb = nc.alloc_sbuf_tensor("wts_sb", [128, 2, 6, 3, 32], f32)
        bias_sb = nc.alloc_sbuf_tensor("bias_sb", [128, 1], f32)
        stg = nc.alloc_sbuf_tensor("stg", [128, 2, r_out, W], f32)
        ps = nc.alloc_psum_tensor("ps", [128, NB, 512], f32)

        # ---- preload ----
        for g in range(G):
            rb, sl, _ = GROUP_POS[g]
            nc.sync.dma_start(wts_sb[rb : rb + 48, sl], wt_d[g])
        nc.sync.dma_start(bias_sb[:, :], bias_d[:, :])
        for g in range(G):
            nc.gpsimd.memset(rings[g][:, :, :], 0.0)

        # ---- input chunk bookkeeping ----
        # issue_map: rho -> list of (g, kh_block, slot_lo, n_slots, row_lo)
        #   row_lo = first input row (None => memset slots)
        issue_map = {r: [] for r in range(h)}

        def emit_chunk(g, blk, sl0, c0, c1, roff):
            """rows [c0+roff, c1+roff) -> block blk slots [sl0 ...)."""
            vlo = min(max(c0, -roff), c1)
            vhi = max(min(c1, h - roff), vlo)
            pieces = []
            if vlo > c0:
                pieces.append((sl0, vlo - c0, None))
            if vhi > vlo:
                pieces.append((sl0 + (vlo - c0), vhi - vlo, vlo + roff))
            if c1 > vhi:
                pieces.append((sl0 + (vhi - c0), c1 - vhi, None))
            return pieces

        for g in range(G):
            d = DIL[g]
            if schemes[g] == "rep":
                for c0 in range(0, h, ch_rep):
                    c1 = min(c0 + ch_rep, h)
                    ip = max(0, c0 - (s_rep - ch_rep))
                    for blk in range(3):
                        roff = (blk - 1) * d
                        for p in emit_chunk(g, blk, c0 % s_rep, c0, c1, roff):
                            issue_map[ip].append((g, blk, *p))
            else:
                # runs: block b, run t covers rows [3dt+db, 3dt+db+d),
                # slot j = row mod d.  Split each run in 2 sub-chunks for
                # WAR slack.  Run (b,t) sub [j0,j1): issue at
                # max(0, base+j1-2d); must land before output base+j0-d.
                t = 0
                while True:
                    base0 = 3 * d * t
                    if base0 >= h + d:
                        break
                    for bidx in range(3):
                        base = base0 + d * bidx
                        if base >= h + d:
                            continue
                        nsub = 2 if d >= 6 else 1
                        step = (d + nsub - 1) // nsub
                        for j0 in range(0, d, step):
                            j1 = min(j0 + step, d)
                            ip = max(0, base + j1 - 2 * d)
                            if ip >= h:
                                continue
                            blk = bidx  # floor(r/d) mod 3 for r in run
                            for p in emit_chunk(g, blk, j0, base + j0, base + j1, 0):
                                issue_map[ip].append((g, blk, *p))
                    t += 1

        # handle mod3 pre-loop "virtual" rows [-d, 0): they live in block
        # (-1) mod 3 = 2, slots [0, d); ring starts memset to zero, so
        # nothing to do (full-ring memset above covers it).

        def ring_part_base(g, blk):
            rb, _, _ = GROUP_POS[g]
            return rb + 16 * blk

        def emit_in_dma(g, blk, sl0, n, row_lo):
            d = DIL[g]
            pb = ring_part_base(g, blk)
            if row_lo is None:
                # stale/zero slots are neutralized by edge weight variants
                return
            dst = rings[g][pb : pb + ICG, sl0 : sl0 + n, d : d + W]
            src = x_d[16 * g : 16 * g + ICG, row_lo : row_lo + n, :]
            nc.sync.dma_start(dst, src)

        # ---- main row loop ----
        act_t = mybir.ActivationFunctionType
        for rho in range(h):
            for item in issue_map[rho]:
                emit_in_dma(*item)
            bank = rho % NB
            for kw in range(3):
                for g in range(G):
                    d = DIL[g]
                    rb, sl, cb = GROUP_POS[g]
                    if schemes[g] == "rep":
                        slot = rho % s_rep
                        v = 1
                    else:
                        slot = rho % d
                        v = (rho // d) % 3
                    if rho >= h - d:
                        v += 3
                    lhsT = wts_sb[rb : rb + 48, sl, v, kw, :]
                    rhs = rings[g][rb : rb + 48, slot, kw * d : kw * d + W]
                    nc.tensor.matmul(
                        ps[cb : cb + 32, bank, 0:W],
                        lhsT,
                        rhs,
                        start=(kw == 0),
                        stop=(kw == 2),
                        tile_position=(rb, cb),
                    )
            half = (rho // r_out) % 2
            src = ps[:, bank, 0:W]
            dst = stg[:, half, rho % r_out, :]
            if rho % 2 == 0:
                nc.scalar.activation(
                    dst, src, act_t.Identity, bias=bias_sb[:, 0:1], scale=1.0
                )
            else:
                nc.vector.tensor_scalar_add(dst, src, bias_sb[:, 0:1])
            if (rho + 1) % r_out == 0:
                r0 = rho + 1 - r_out
                for g in range(G):
                    _, _, cb = GROUP_POS[g]
                    nc.scalar.dma_start(
                        out_d[16 * g : 16 * g + OCG, r0 : r0 + r_out, :],
                        stg[cb : cb + OCG, half, :, :],
                    )

    nc.compile()
    return nc


_NC_CACHE = {}


def _get_nc(**kw):
    key = tuple(sorted(kw.items()))
    if key not in _NC_CACHE:
        _NC_CACHE[key] = build_module(**kw)
    return _NC_CACHE[key]


def kernel(x: np.ndarray, weight: np.ndarray, bias: np.ndarray, *, trace=False):
    from concourse.bass_utils import run_bass_kernel_spmd

    assert x.shape == (B, C, H, W), x.shape
    nc = _get_nc()
    wt = _prep_weights(np.asarray(weight, np.float32), SCHEMES)
    bp = _prep_bias(np.asarray(bias, np.float32))
    xs = np.ascontiguousarray(np.asarray(x, np.float32))
    in_maps = [
        {"x": xs[i], "wt": wt, "biasp": bp} for i in range(NCORES)
    ]
    res = run_bass_kernel_spmd(nc, in_maps, list(range(NCORES)), trace=trace)
    out = np.stack([res.results[i]["out"] for i in range(NCORES)], axis=0)
    if trace:
        kernel.last_exec_time_ns = res.exec_time_ns
        kernel.last_results = res
    return out


# revision 7
# speedup vs baseline: 1.2233x; 1.2233x over previous
"""Merged dilated-group conv2d (4 groups, dil 1/6/12/18) for TRN2, 8 cores.

Sharding: data-parallel over batch (8 images -> 8 cores).

Per-core kernel:
  conv-as-matmul, contraction K = 48 = (3 kh-taps x 16 in-ch) on SBUF
  partitions; the 3 kw-taps are PSUM-accumulating matmuls reading the same
  SBUF row-window at shifted free-dim offsets. All 4 dilation groups run
  concurrently on disjoint PE subarray quadrants via tile_position
  (row-tiles {0,64} x col-tiles {0,32,64,96}); M=32 per group (16 real
  out-channels + 16 zero cols) so the 4 groups tile a PSUM bank across all
  128 partitions and each output row is evicted with a single
  128-partition op (bias fused), alternating ACT/DVE.

  Input rows stream through per-group SBUF rings of zero-padded row
  windows (Wp = W + 2d).  Two ring schemes per group:
    'rep'  : slot per output row holds the 3 kh rows (rows read from HBM
             3x, deep ring, fully decoupled pipelining)
    'mod3' : row r lives once in partition block floor(r/d) mod 3, slot
             r mod d; the kh->block rotation is absorbed into 3
             precomputed weight variants (rows read from HBM 1x)
"""

import os
import numpy as np

H = 320
W = 320
B = 8
C = 64
G = 4
ICG = 16  # in-channels per group
OCG = 16  # out-channels per group
DIL = (1, 6, 12, 18)
NCORES = 8

# group g -> (row-tile base RB, slot s within row tile, col-tile base)
GROUP_POS = {
    0: (0, 0, 0),
    1: (64, 0, 32),
    2: (0, 1, 64),
    3: (64, 1, 96),
}

# per-group input scheme: 'rep' (3x HBM reads) or 'mod3' (1x HBM reads)
SCHEMES = ("rep", "mod3", "mod3", "mod3")

S_REP = 16   # rep-ring slots (must be multiple of CH_REP)
CH_REP = 8   # rep-ring DMA chunk (output rows per in-DMA)
DT_IN = "float16"  # matmul operand dtype: float32 | float16 | bfloat16
R_OUT = 16   # output staging rows per out-DMA block
NB = 8       # psum banks in rotation


def _np_dt(dt_in):
    import ml_dtypes

    return {"float32": np.float32, "float16": np.float16,
            "bfloat16": ml_dtypes.bfloat16}[dt_in]


def _prep_weights(weight: np.ndarray, schemes, dt_in=DT_IN) -> np.ndarray:
    """-> [G, 48, 3(variant), 3(kw), 32] lhsT tiles.

    lhsT row q = 16*b + ic (b = partition block), col j = out-channel
    (j<16) or zero pad (j>=16).  Block b holds kh = b for 'rep'; for
    'mod3' output row rho with v = floor(rho/d) mod 3 reads kh =
    (b - v + 1) mod 3 from block b.
    """
    wt = np.zeros((G, 48, 6, 3, 32), _np_dt(dt_in))
    for g in range(G):
        for v in range(3):
            for b in range(3):
                kh = b if schemes[g] == "rep" else (b - v + 1) % 3
                for ic in range(ICG):
                    # weight[oc_global, ic, kh, kw] -> tile[16b+ic, v, kw, oc]
                    wt[g, 16 * b + ic, v, :, :OCG] = weight[
                        16 * g : 16 * g + OCG, ic, kh, :
                    ].T
            # bottom-edge variant v+3: kh=2 block zeroed (reads stale slots)
            bstar = 2 if schemes[g] == "rep" else (v + 1) % 3
            wt[g, :, v + 3] = wt[g, :, v]
            wt[g, 16 * bstar : 16 * bstar + ICG, v + 3] = 0.0
    return wt


def _prep_bias(bias: np.ndarray) -> np.ndarray:
    bp = np.zeros((128, 1), np.float32)
    for g in range(G):
        _, _, cb = GROUP_POS[g]
        bp[cb : cb + OCG, 0] = bias[16 * g : 16 * g + OCG]
    return bp


def build_module(h=H, schemes=SCHEMES, s_rep=S_REP, ch_rep=CH_REP, r_out=R_OUT,
                 dt_in=DT_IN):
    import concourse.bass as bass
    import concourse.tile as tile
    from concourse import bacc, mybir

    f32 = mybir.dt.float32
    fin = getattr(mybir.dt, dt_in)
    Wp = [W + 2 * d for d in DIL]

    nc = bacc.Bacc("TRN2", target_bir_lowering=False, debug=False)
    x_d = nc.dram_tensor("x", [C, h, W], f32, kind="ExternalInput")
    wt_d = nc.dram_tensor("wt", [G, 48, 6, 3, 32], fin, kind="ExternalInput")
    bias_d = nc.dram_tensor("biasp", [128, 1], f32, kind="ExternalInput")
    out_d = nc.dram_tensor("out", [C, h, W], f32, kind="ExternalOutput")

    with tile.TileContext(nc) as tc:
        # ---- persistent SBUF/PSUM ----
        rings = []
        for g in range(G):
            nslot = s_rep if schemes[g] == "rep" else DIL[g]
            rings.append(
                nc.alloc_sbuf_tensor(f"ring{g}", [128, nslot, Wp[g]], fin)
            )
        wts_sb = nc.alloc_sbuf_tensor("wts_sb", [128, 2, 6, 3, 32], fin)
        bias_sb = nc.alloc_sbuf_tensor("bias_sb", [128, 1], f32)
        stg = nc.alloc_sbuf_tensor("stg", [128, 2, r_out, W], f32)
        ps = nc.alloc_psum_tensor("ps", [128, NB, 512], f32)

        # ---- preload ----
        for g in range(G):
            rb, sl, _ = GROUP_POS[g]
            nc.sync.dma_start(wts_sb[rb : rb + 48, sl], wt_d[g])
        nc.sync.dma_start(bias_sb[:, :], bias_d[:, :])
        for g in range(G):
            nc.gpsimd.memset(rings[g][:, :, :], 0.0)

        # ---- input chunk bookkeeping ----
        # issue_map: rho -> list of (g, kh_block, slot_lo, n_slots, row_lo)
        #   row_lo = first input row (None => memset slots)
        issue_map = {r: [] for r in range(h)}

        def emit_chunk(g, blk, sl0, c0, c1, roff):
            """rows [c0+roff, c1+roff) -> block blk slots [sl0 ...)."""
            vlo = min(max(c0, -roff), c1)
            vhi = max(min(c1, h - roff), vlo)
            pieces = []
            if vlo > c0:
                pieces.append((sl0, vlo - c0, None))
            if vhi > vlo:
                pieces.append((sl0 + (vlo - c0), vhi - vlo, vlo + roff))
            if c1 > vhi:
                pieces.append((sl0 + (vhi - c0), c1 - vhi, None))
            return pieces

        for g in range(G):
            d = DIL[g]
            if schemes[g] == "rep":
                for c0 in range(0, h, ch_rep):
                    c1 = min(c0 + ch_rep, h)
                    ip = max(0, c0 - (s_rep - ch_rep))
                    for blk in range(3):
                        roff = (blk - 1) * d
                        for p in emit_chunk(g, blk, c0 % s_rep, c0, c1, roff):
                            issue_map[ip].append((g, blk, *p))
            else:
                # runs: block b, run t covers rows [3dt+db, 3dt+db+d),
                # slot j = row mod d.  Split each run in 2 sub-chunks for
                # WAR slack.  Run (b,t) sub [j0,j1): issue at
                # max(0, base+j1-2d); must land before output base+j0-d.
                t = 0
                while True:
                    base0 = 3 * d * t
                    if base0 >= h + d:
                        break
                    for bidx in range(3):
                        base = base0 + d * bidx
                        if base >= h + d:
                            continue
                        nsub = 2 if d >= 6 else 1
                        step = (d + nsub - 1) // nsub
                        for j0 in range(0, d, step):
                            j1 = min(j0 + step, d)
                            ip = max(0, base + j1 - 2 * d)
                            if ip >= h:
                                continue
                            blk = bidx  # floor(r/d) mod 3 for r in run
                            for p in emit_chunk(g, blk, j0, base + j0, base + j1, 0):
                                issue_map[ip].append((g, blk, *p))
                    t += 1

        # handle mod3 pre-loop "virtual" rows [-d, 0): they live in block
        # (-1) mod 3 = 2, slots [0, d); ring starts memset to zero, so
        # nothing to do (full-ring memset above covers it).

        def ring_part_base(g, blk):
            rb, _, _ = GROUP_POS[g]
            return rb + 16 * blk

        def emit_in_dma(g, blk, sl0, n, row_lo):
            d = DIL[g]
            pb = ring_part_base(g, blk)
            if row_lo is None:
                # stale/zero slots are neutralized by edge weight variants
                return
            dst = rings[g][pb : pb + ICG, sl0 : sl0 + n, d : d + W]
            src = x_d[16 * g : 16 * g + ICG, row_lo : row_lo + n, :]
            if dt_in == "float32":
                nc.sync.dma_start(dst, src)
            else:
                nc.gpsimd.dma_start(dst, src)  # SWDGE casting DMA

        # ---- main row loop ----
        act_t = mybir.ActivationFunctionType
        for rho in range(h):
            for item in issue_map[rho]:
                emit_in_dma(*item)
            bank = rho % NB
            for kw in range(3):
                for g in range(G):
                    d = DIL[g]
                    rb, sl, cb = GROUP_POS[g]
                    if schemes[g] == "rep":
                        slot = rho % s_rep
                        v = 1
                    else:
                        slot = rho % d
                        v = (rho // d) % 3
                    if rho >= h - d:
                        v += 3
                    lhsT = wts_sb[rb : rb + 48, sl, v, kw, :]
                    rhs = rings[g][rb : rb + 48, slot, kw * d : kw * d + W]
                    nc.tensor.matmul(
                        ps[cb : cb + 32, bank, 0:W],
                        lhsT,
                        rhs,
                        start=(kw == 0),
                        stop=(kw == 2),
                        tile_position=(rb, cb),
                    )
            half = (rho // r_out) % 2
            src = ps[:, bank, 0:W]
            dst = stg[:, half, rho % r_out, :]
            if rho % 2 == 0:
                nc.scalar.activation(
                    dst, src, act_t.Identity, bias=bias_sb[:, 0:1], scale=1.0
                )
            else:
                nc.vector.tensor_scalar_add(dst, src, bias_sb[:, 0:1])
            if (rho + 1) % r_out == 0:
                r0 = rho + 1 - r_out
                for g in range(G):
                    _, _, cb = GROUP_POS[g]
                    nc.scalar.dma_start(
                        out_d[16 * g : 16 * g + OCG, r0 : r0 + r_out, :],
                        stg[cb : cb + OCG, half, :, :],
                    )

    nc.compile()
    return nc


_NC_CACHE = {}


def _get_nc(**kw):
    key = tuple(sorted(kw.items()))
    if key not in _NC_CACHE:
        _NC_CACHE[key] = build_module(**kw)
    return _NC_CACHE[key]


def kernel(x: np.ndarray, weight: np.ndarray, bias: np.ndarray, *, trace=False):
    from concourse.bass_utils import run_bass_kernel_spmd

    assert x.shape == (B, C, H, W), x.shape
    nc = _get_nc()
    wt = _prep_weights(np.asarray(weight, np.float32), SCHEMES, DT_IN)
    bp = _prep_bias(np.asarray(bias, np.float32))
    xs = np.ascontiguousarray(np.asarray(x, np.float32))
    in_maps = [
        {"x": xs[i], "wt": wt, "biasp": bp} for i in range(NCORES)
    ]
    res = run_bass_kernel_spmd(nc, in_maps, list(range(NCORES)), trace=trace)
    out = np.stack([res.results[i]["out"] for i in range(NCORES)], axis=0)
    if trace:
        kernel.last_exec_time_ns = res.exec_time_ns
        kernel.last_results = res
    return out


# revision 8
# speedup vs baseline: 1.4362x; 1.1741x over previous
"""Merged dilated-group conv2d (4 groups, dil 1/6/12/18) for TRN2, 8 cores.

Sharding: data-parallel over batch (8 images -> 8 cores).

Per-core kernel:
  conv-as-matmul, contraction K = 48 = (3 kh-taps x 16 in-ch) on SBUF
  partitions; the 3 kw-taps are PSUM-accumulating matmuls reading the same
  SBUF row-window at shifted free-dim offsets. All 4 dilation groups run
  concurrently on disjoint PE subarray quadrants via tile_position
  (row-tiles {0,64} x col-tiles {0,32,64,96}); M=32 per group (16 real
  out-channels + 16 zero cols) so the 4 groups tile a PSUM bank across all
  128 partitions and each output row is evicted with a single
  128-partition op (bias fused), alternating ACT/DVE.

  Input rows stream through per-group SBUF rings of zero-padded row
  windows (Wp = W + 2d).  Two ring schemes per group:
    'rep'  : slot per output row holds the 3 kh rows (rows read from HBM
             3x, deep ring, fully decoupled pipelining)
    'mod3' : row r lives once in partition block floor(r/d) mod 3, slot
             r mod d; the kh->block rotation is absorbed into 3
             precomputed weight variants (rows read from HBM 1x)
"""

import os
import numpy as np

H = 320
W = 320
B = 8
C = 64
G = 4
ICG = 16  # in-channels per group
OCG = 16  # out-channels per group
DIL = (1, 6, 12, 18)
NCORES = 8

# group g -> (row-tile base RB, slot s within row tile, col-tile base)
GROUP_POS = {
    0: (0, 0, 0),
    1: (64, 0, 32),
    2: (0, 1, 64),
    3: (64, 1, 96),
}

# per-group input scheme: 'rep' (3x HBM reads) or 'mod3' (1x HBM reads)
SCHEMES = ("rep", "mod3", "mod3", "mod3")

S_REP = 32   # rep-ring slots (must be multiple of CH_REP)
CH_REP = 16  # rep-ring DMA chunk (output rows per in-DMA)
DT_IN = "float16"  # matmul operand dtype: float32 | float16 | bfloat16
R_OUT = 16   # output staging rows per out-DMA block
NB = 8       # psum banks in rotation


def _np_dt(dt_in):
    import ml_dtypes

    return {"float32": np.float32, "float16": np.float16,
            "bfloat16": ml_dtypes.bfloat16}[dt_in]


def _prep_weights(weight: np.ndarray, schemes, dt_in=DT_IN) -> np.ndarray:
    """-> [G, 48, 3(variant), 3(kw), 32] lhsT tiles.

    lhsT row q = 16*b + ic (b = partition block), col j = out-channel
    (j<16) or zero pad (j>=16).  Block b holds kh = b for 'rep'; for
    'mod3' output row rho with v = floor(rho/d) mod 3 reads kh =
    (b - v + 1) mod 3 from block b.
    """
    wt = np.zeros((G, 48, 6, 3, 32), _np_dt(dt_in))
    for g in range(G):
        for v in range(3):
            for b in range(3):
                kh = b if schemes[g] == "rep" else (b - v + 1) % 3
                for ic in range(ICG):
                    # weight[oc_global, ic, kh, kw] -> tile[16b+ic, v, kw, oc]
                    wt[g, 16 * b + ic, v, :, :OCG] = weight[
                        16 * g : 16 * g + OCG, ic, kh, :
                    ].T
            # bottom-edge variant v+3: kh=2 block zeroed (reads stale slots)
            bstar = 2 if schemes[g] == "rep" else (v + 1) % 3
            wt[g, :, v + 3] = wt[g, :, v]
            wt[g, 16 * bstar : 16 * bstar + ICG, v + 3] = 0.0
    return wt


def _prep_bias(bias: np.ndarray) -> np.ndarray:
    bp = np.zeros((128, 1), np.float32)
    for g in range(G):
        _, _, cb = GROUP_POS[g]
        bp[cb : cb + OCG, 0] = bias[16 * g : 16 * g + OCG]
    return bp


def build_module(h=H, schemes=SCHEMES, s_rep=S_REP, ch_rep=CH_REP, r_out=R_OUT,
                 dt_in=DT_IN):
    import concourse.bass as bass
    import concourse.tile as tile
    from concourse import bacc, mybir

    f32 = mybir.dt.float32
    fin = getattr(mybir.dt, dt_in)
    Wp = [W + 2 * d for d in DIL]

    nc = bacc.Bacc("TRN2", target_bir_lowering=False, debug=False)
    x_d = nc.dram_tensor("x", [C, h, W], f32, kind="ExternalInput")
    wt_d = nc.dram_tensor("wt", [G, 48, 6, 3, 32], fin, kind="ExternalInput")
    bias_d = nc.dram_tensor("biasp", [128, 1], f32, kind="ExternalInput")
    out_d = nc.dram_tensor("out", [C, h, W], f32, kind="ExternalOutput")

    with tile.TileContext(nc) as tc:
        # ---- persistent SBUF/PSUM ----
        rings = []
        for g in range(G):
            nslot = s_rep if schemes[g] == "rep" else DIL[g]
            rings.append(
                nc.alloc_sbuf_tensor(f"ring{g}", [128, nslot, Wp[g]], fin)
            )
        wts_sb = nc.alloc_sbuf_tensor("wts_sb", [128, 2, 6, 3, 32], fin)
        bias_sb = nc.alloc_sbuf_tensor("bias_sb", [128, 1], f32)
        stg = nc.alloc_sbuf_tensor("stg", [128, 2, r_out, W], f32)
        ps = nc.alloc_psum_tensor("ps", [128, NB, 512], f32)

        # ---- preload ----
        for g in range(G):
            rb, sl, _ = GROUP_POS[g]
            nc.sync.dma_start(wts_sb[rb : rb + 48, sl], wt_d[g])
        nc.sync.dma_start(bias_sb[:, :], bias_d[:, :])
        for g in range(G):
            nc.gpsimd.memset(rings[g][:, :, :], 0.0)

        # ---- input chunk bookkeeping ----
        # issue_map: rho -> list of (g, kh_block, slot_lo, n_slots, row_lo)
        #   row_lo = first input row (None => memset slots)
        issue_map = {r: [] for r in range(h)}

        def emit_chunk(g, blk, sl0, c0, c1, roff):
            """rows [c0+roff, c1+roff) -> block blk slots [sl0 ...)."""
            vlo = min(max(c0, -roff), c1)
            vhi = max(min(c1, h - roff), vlo)
            pieces = []
            if vlo > c0:
                pieces.append((sl0, vlo - c0, None))
            if vhi > vlo:
                pieces.append((sl0 + (vlo - c0), vhi - vlo, vlo + roff))
            if c1 > vhi:
                pieces.append((sl0 + (vhi - c0), c1 - vhi, None))
            return pieces

        for g in range(G):
            d = DIL[g]
            if schemes[g] == "rep":
                for c0 in range(0, h, ch_rep):
                    c1 = min(c0 + ch_rep, h)
                    ip = max(0, c0 - (s_rep - ch_rep))
                    for blk in range(3):
                        roff = (blk - 1) * d
                        for p in emit_chunk(g, blk, c0 % s_rep, c0, c1, roff):
                            issue_map[ip].append((g, blk, *p))
            else:
                # runs: block b, run t covers rows [3dt+db, 3dt+db+d),
                # slot j = row mod d.  Split each run in 2 sub-chunks for
                # WAR slack.  Run (b,t) sub [j0,j1): issue at
                # max(0, base+j1-2d); must land before output base+j0-d.
                t = 0
                while True:
                    base0 = 3 * d * t
                    if base0 >= h + d:
                        break
                    for bidx in range(3):
                        base = base0 + d * bidx
                        if base >= h + d:
                            continue
                        nsub = 2 if d >= 6 else 1
                        step = (d + nsub - 1) // nsub
                        for j0 in range(0, d, step):
                            j1 = min(j0 + step, d)
                            ip = max(0, base + j1 - 2 * d)
                            if ip >= h:
                                continue
                            blk = bidx  # floor(r/d) mod 3 for r in run
                            for p in emit_chunk(g, blk, j0, base + j0, base + j1, 0):
                                issue_map[ip].append((g, blk, *p))
                    t += 1

        # handle mod3 pre-loop "virtual" rows [-d, 0): they live in block
        # (-1) mod 3 = 2, slots [0, d); ring starts memset to zero, so
        # nothing to do (full-ring memset above covers it).

        def ring_part_base(g, blk):
            rb, _, _ = GROUP_POS[g]
            return rb + 16 * blk

        def emit_in_dma(g, blk, sl0, n, row_lo):
            d = DIL[g]
            pb = ring_part_base(g, blk)
            if row_lo is None:
                # stale/zero slots are neutralized by edge weight variants
                return
            dst = rings[g][pb : pb + ICG, sl0 : sl0 + n, d : d + W]
            src = x_d[16 * g : 16 * g + ICG, row_lo : row_lo + n, :]
            if dt_in == "float32":
                nc.sync.dma_start(dst, src)
            else:
                nc.gpsimd.dma_start(dst, src)  # SWDGE casting DMA

        # ---- main row loop ----
        act_t = mybir.ActivationFunctionType
        for rho in range(h):
            for item in issue_map[rho]:
                emit_in_dma(*item)
            bank = rho % NB
            for kw in range(3):
                for g in range(G):
                    d = DIL[g]
                    rb, sl, cb = GROUP_POS[g]
                    if schemes[g] == "rep":
                        slot = rho % s_rep
                        v = 1
                    else:
                        slot = rho % d
                        v = (rho // d) % 3
                    if rho >= h - d:
                        v += 3
                    lhsT = wts_sb[rb : rb + 48, sl, v, kw, :]
                    rhs = rings[g][rb : rb + 48, slot, kw * d : kw * d + W]
                    nc.tensor.matmul(
                        ps[cb : cb + 32, bank, 0:W],
                        lhsT,
                        rhs,
                        start=(kw == 0),
                        stop=(kw == 2),
                        tile_position=(rb, cb),
                    )
            half = (rho // r_out) % 2
            src = ps[:, bank, 0:W]
            dst = stg[:, half, rho % r_out, :]
            if rho % 2 == 0:
                nc.scalar.activation(
                    dst, src, act_t.Identity, bias=bias_sb[:, 0:1], scale=1.0
                )
            else:
                nc.vector.tensor_scalar_add(dst, src, bias_sb[:, 0:1])
            if (rho + 1) % r_out == 0:
                r0 = rho + 1 - r_out
                for g in range(G):
                    _, _, cb = GROUP_POS[g]
                    nc.sync.dma_start(
                        out_d[16 * g : 16 * g + OCG, r0 : r0 + r_out, :],
                        stg[cb : cb + OCG, half, :, :],
                    )

    nc.compile()
    return nc


_NC_CACHE = {}


def _get_nc(**kw):
    key = tuple(sorted(kw.items()))
    if key not in _NC_CACHE:
        _NC_CACHE[key] = build_module(**kw)
    return _NC_CACHE[key]


def kernel(x: np.ndarray, weight: np.ndarray, bias: np.ndarray, *, trace=False):
    from concourse.bass_utils import run_bass_kernel_spmd

    assert x.shape == (B, C, H, W), x.shape
    nc = _get_nc()
    wt = _prep_weights(np.asarray(weight, np.float32), SCHEMES, DT_IN)
    bp = _prep_bias(np.asarray(bias, np.float32))
    xs = np.ascontiguousarray(np.asarray(x, np.float32))
    in_maps = [
        {"x": xs[i], "wt": wt, "biasp": bp} for i in range(NCORES)
    ]
    res = run_bass_kernel_spmd(nc, in_maps, list(range(NCORES)), trace=trace)
    out = np.stack([res.results[i]["out"] for i in range(NCORES)], axis=0)
    if trace:
        kernel.last_exec_time_ns = res.exec_time_ns
        kernel.last_results = res
    return out
